# revision 1
# baseline (speedup 1.0000x reference)
"""Chamfer distance kernel for Trainium2 (8 NeuronCores, SPMD).

Problem: x, y ~ [4, 8192, 3] f32.  Output: scalar f32
    mean_i min_j ||x_i - y_j||^2  +  mean_j min_i ||x_i - y_j||^2
(means over batch*8192).

Strategy
--------
Shard (batch b, x-row half h) across the 8 cores: core c = 2*b + h owns
x[b, h*4096:(h+1)*4096] vs all of y[b].

Each core computes the 4096x8192 block of NEGATED squared distances with a
single K=16 bf16 matmul per tile:
    -dist[i,j] = -x2[i] - y2[j] + 2*sum_d x[i,d]*y[j,d]
where every f32 factor is split into bf16 hi+lo parts (4 K-rows for the
norm terms, 12 for the 3 coordinate products) so precision is ~fp32 while
the PE runs at bf16 speed (1 cycle/row vs 4 for f32).

Negation turns both min-reductions into max-reductions:
  * row direction (min over j): free-axis reduce_max per PSUM tile
  * col direction (min over i): elementwise running max into an SBUF
    accumulator [128, 8192], then a cross-partition max (gpsimd) at the end.

Host combines: row maxes are complete per core (sum them); column partial
maxes are elementwise-maxed between the two cores sharing a batch, then
summed.  All final means in f64, returned as f32.
"""

import numpy as np
import ml_dtypes

import concourse.bacc as bacc
import concourse.bass as bass
import concourse.mybir as mybir
import concourse.tile as tile
from concourse import bass_isa
from concourse.bass_utils import run_bass_kernel_spmd

BF16 = ml_dtypes.bfloat16

B = 4
N = 8192
D = 3
HALF = N // 2           # x-rows per core
NCORES = 8
K = 16                  # augmented contraction dim
MBLK = 128              # x-rows per block (PSUM partition dim)
NBLKS = HALF // MBLK    # 32
PSUM_W = 2048           # psum tile width (4 f32 banks)
NPS = N // PSUM_W       # 4 psum tiles per row block
MM_N = 512              # matmul free width (1 f32 psum bank)

_NC_CACHE = None


def _build_nc():
    global _NC_CACHE
    if _NC_CACHE is not None:
        return _NC_CACHE

    nc = bacc.Bacc("TRN2", target_bir_lowering=False, debug=False,
                   num_devices=NCORES)
    lhs_d = nc.dram_tensor("lhst", [K, HALF], mybir.dt.bfloat16,
                           kind="ExternalInput")
    rhs_d = nc.dram_tensor("rhs", [K, N], mybir.dt.bfloat16,
                           kind="ExternalInput")
    row_d = nc.dram_tensor("rowout", [MBLK, NBLKS], mybir.dt.float32,
                           kind="ExternalOutput")
    col_d = nc.dram_tensor("colout", [1, N], mybir.dt.float32,
                           kind="ExternalOutput")

    with tile.TileContext(nc) as tc:
        with tc.tile_pool(name="sb", bufs=1) as sb, \
             tc.tile_pool(name="ps", bufs=2, space=bass.MemorySpace.PSUM) as ps:
            lhs_sb = sb.tile([K, HALF], mybir.dt.bfloat16)
            rhs_sb = sb.tile([K, N], mybir.dt.bfloat16)
            nc.sync.dma_start(lhs_sb[:], lhs_d.ap()[:])
            nc.sync.dma_start(rhs_sb[:], rhs_d.ap()[:])

            colacc = sb.tile([MBLK, N], mybir.dt.float32)
            rowpart = sb.tile([MBLK, NBLKS * NPS], mybir.dt.float32)
            rowcol = sb.tile([MBLK, NBLKS], mybir.dt.float32)

            for ib in range(NBLKS):
                w = lhs_sb[:, ib * MBLK:(ib + 1) * MBLK]
                for jp in range(NPS):
                    pt = ps.tile([MBLK, PSUM_W], mybir.dt.float32)
                    for m in range(PSUM_W // MM_N):
                        j0 = jp * PSUM_W + m * MM_N
                        nc.tensor.matmul(pt[:, m * MM_N:(m + 1) * MM_N], w,
                                         rhs_sb[:, j0:j0 + MM_N],
                                         start=True, stop=True)
                    csl = colacc[:, jp * PSUM_W:(jp + 1) * PSUM_W]
                    if ib == 0:
                        nc.vector.tensor_copy(out=csl, in_=pt[:])
                    else:
                        nc.vector.tensor_tensor(out=csl, in0=csl, in1=pt[:],
                                                op=mybir.AluOpType.max)
                    slot = ib * NPS + jp
                    nc.vector.tensor_reduce(
                        out=rowpart[:, slot:slot + 1], in_=pt[:],
                        axis=mybir.AxisListType.X, op=mybir.AluOpType.max)

            nc.vector.tensor_reduce(
                out=rowcol[:],
                in_=rowpart[:].rearrange("p (a b) -> p a b", b=NPS),
                axis=mybir.AxisListType.X, op=mybir.AluOpType.max)

            colred = sb.tile([MBLK, N], mybir.dt.float32)
            nc.gpsimd.partition_all_reduce(colred[:], colacc[:], channels=MBLK,
                                           reduce_op=bass_isa.ReduceOp.max)

            nc.sync.dma_start(row_d.ap()[:], rowcol[:])
            nc.sync.dma_start(col_d.ap()[:], colred[0:1, :])

    nc.compile()
    _NC_CACHE = nc
    return nc


def _split(v):
    """f32 -> (hi, lo) bf16 with v ~= hi + lo to ~18 mantissa bits."""
    hi = v.astype(BF16)
    lo = (v - hi.astype(np.float32)).astype(BF16)
    return hi, lo


def _prep_core(xc, yb):
    """Build the K=16 augmented bf16 operands for one core.

    xc: [HALF, 3] f32 x-chunk; yb: [N, 3] f32 full y for the batch.
    Row k of lhs/rhs multiply pairwise and accumulate so that
    psum[i, j] = -dist2(x_i, y_j) to ~1e-5 abs.
    """
    x2 = np.sum(xc * xc, axis=1)
    y2 = np.sum(yb * yb, axis=1)
    nx2h, nx2l = _split(-x2)
    ny2h, ny2l = _split(-y2)
    xh, xl = _split(xc)
    yh, yl = _split(yb)
    txh = (xh.astype(np.float32) * 2.0).astype(BF16)  # exact in bf16
    txl = (xl.astype(np.float32) * 2.0).astype(BF16)

    lhs = np.zeros((K, HALF), dtype=BF16)
    rhs = np.zeros((K, N), dtype=BF16)
    lhs[0] = nx2h
    lhs[1] = nx2l
    rhs[0] = np.ones(N, BF16)
    rhs[1] = np.ones(N, BF16)
    lhs[2] = np.ones(HALF, BF16)
    lhs[3] = np.ones(HALF, BF16)
    rhs[2] = ny2h
    rhs[3] = ny2l
    for d in range(D):
        lhs[4 + d] = txh[:, d]
        rhs[4 + d] = yh[:, d]
        lhs[7 + d] = txl[:, d]
        rhs[7 + d] = yh[:, d]
        lhs[10 + d] = txh[:, d]
        rhs[10 + d] = yl[:, d]
        lhs[13 + d] = txl[:, d]
        rhs[13 + d] = yl[:, d]
    return {"lhst": lhs, "rhs": rhs}


def make_in_maps(x, y):
    x = np.ascontiguousarray(np.asarray(x, dtype=np.float32))
    y = np.ascontiguousarray(np.asarray(y, dtype=np.float32))
    in_maps = []
    for c in range(NCORES):
        b, h = c // 2, c % 2
        in_maps.append(_prep_core(x[b, h * HALF:(h + 1) * HALF], y[b]))
    return in_maps


def combine(results):
    """results: list of 8 dicts with 'rowout' [128, 32] and 'colout' [1, N],
    both holding NEGATED maxes (i.e. -min distances)."""
    row_sum = 0.0
    col_sum = 0.0
    for b in range(B):
        r0 = results[2 * b]["rowout"].astype(np.float64)
        r1 = results[2 * b + 1]["rowout"].astype(np.float64)
        row_sum += -(r0.sum() + r1.sum())
        c0 = results[2 * b]["colout"][0]
        c1 = results[2 * b + 1]["colout"][0]
        col_sum += -np.maximum(c0, c1).astype(np.float64).sum()
    mean_x = row_sum / (B * N)
    mean_y = col_sum / (B * N)
    return np.asarray(mean_x + mean_y, dtype=np.float32)


def kernel(x, y):
    nc = _build_nc()
    in_maps = make_in_maps(x, y)
    res = run_bass_kernel_spmd(nc, in_maps, core_ids=list(range(NCORES)))
    return combine(res.results)


# revision 3
# speedup vs baseline: 1.6084x; 1.6084x over previous
"""Chamfer distance kernel for Trainium2 (8 NeuronCores, SPMD).

Problem: x, y ~ [4, 8192, 3] f32.  Output: scalar f32
    mean_i min_j ||x_i - y_j||^2  +  mean_j min_i ||x_i - y_j||^2
(means over batch*8192).

Strategy
--------
Shard (batch b, x-row half h) across the 8 cores: core c = 2*b + h owns
x[b, h*4096:(h+1)*4096] vs all of y[b].

Each core computes the 4096x8192 block of NEGATED squared distances with a
single K=16 bf16 matmul per tile:
    -dist[i,j] = -x2[i] - y2[j] + 2*sum_d x[i,d]*y[j,d]
where every f32 factor is split into bf16 hi+lo parts (4 K-rows for the
norm terms, 12 for the 3 coordinate products) so precision is ~fp32 while
the PE runs at bf16 speed (1 cycle/row vs 4 for f32).

Negation turns both min-reductions into max-reductions:
  * row direction (min over j): free-axis reduce_max per PSUM tile
  * col direction (min over i): elementwise running max into an SBUF
    accumulator [128, 8192], then a cross-partition max (gpsimd) at the end.

Host combines: row maxes are complete per core (sum them); column partial
maxes are elementwise-maxed between the two cores sharing a batch, then
summed.  All final means in f64, returned as f32.
"""

import numpy as np
import ml_dtypes

import concourse.bacc as bacc
import concourse.bass as bass
import concourse.mybir as mybir
import concourse.tile as tile
from concourse import bass_isa
from concourse.bass_utils import run_bass_kernel_spmd

BF16 = ml_dtypes.bfloat16

B = 4
N = 8192
D = 3
HALF = N // 2           # x-rows per core
NCORES = 8
K = 16                  # augmented contraction dim
MBLK = 128              # x-rows per block (PSUM partition dim)
NBLKS = HALF // MBLK    # 32
PSUM_W = 2048           # psum tile width (4 f32 banks)
NPS = N // PSUM_W       # 4 psum tiles per row block
MM_N = 512              # matmul free width (1 f32 psum bank)

_NC_CACHE = None


def _build_nc():
    global _NC_CACHE
    if _NC_CACHE is not None:
        return _NC_CACHE

    nc = bacc.Bacc("TRN2", target_bir_lowering=False, debug=False,
                   num_devices=NCORES)
    lhs_d = nc.dram_tensor("lhst", [K, HALF], mybir.dt.bfloat16,
                           kind="ExternalInput")
    rhs_d = nc.dram_tensor("rhs", [K, N], mybir.dt.bfloat16,
                           kind="ExternalInput")
    row_d = nc.dram_tensor("rowout", [MBLK, NBLKS], mybir.dt.float32,
                           kind="ExternalOutput")
    col_d = nc.dram_tensor("colout", [1, N], mybir.dt.float32,
                           kind="ExternalOutput")

    with tile.TileContext(nc) as tc:
        with tc.tile_pool(name="sb", bufs=1) as sb, \
             tc.tile_pool(name="ps", bufs=2, space=bass.MemorySpace.PSUM) as ps:
            lhs_sb = sb.tile([K, HALF], mybir.dt.bfloat16)
            rhs_sb = sb.tile([K, N], mybir.dt.bfloat16)
            nc.sync.dma_start(lhs_sb[:], lhs_d.ap()[:])
            nc.sync.dma_start(rhs_sb[:], rhs_d.ap()[:])

            # fp16 post-matmul pipeline: ACT casts each PSUM tile to fp16 in
            # SBUF; every elementwise max on DVE then runs in 2x_1p mode.
            # Distances of interest (the maxima of negdist, i.e. values near
            # zero) keep ~1e-5 abs precision in fp16; only far pairs (which
            # lose every max anyway) are coarse.
            colacc = sb.tile([MBLK, N], mybir.dt.float16)
            rowscr = sb.tile([MBLK, PSUM_W], mybir.dt.float16)
            rowhalf = sb.tile([MBLK, PSUM_W // 2], mybir.dt.float16)
            rowcol = sb.tile([MBLK, NBLKS], mybir.dt.float32)

            with tc.tile_pool(name="stage", bufs=4) as stg:
                for ib in range(NBLKS):
                    w = lhs_sb[:, ib * MBLK:(ib + 1) * MBLK]
                    for jp in range(NPS):
                        pt = ps.tile([MBLK, PSUM_W], mybir.dt.float32)
                        for m in range(PSUM_W // MM_N):
                            j0 = jp * PSUM_W + m * MM_N
                            nc.tensor.matmul(pt[:, m * MM_N:(m + 1) * MM_N],
                                             w, rhs_sb[:, j0:j0 + MM_N],
                                             start=True, stop=True)
                        st = stg.tile([MBLK, PSUM_W], mybir.dt.float16,
                                      tag="stage")
                        nc.scalar.copy(out=st[:], in_=pt[:])
                        # column direction: running elementwise max over ib
                        csl = colacc[:, jp * PSUM_W:(jp + 1) * PSUM_W]
                        if ib == 0:
                            nc.vector.tensor_copy(out=csl, in_=st[:])
                        else:
                            nc.vector.tensor_tensor(out=csl, in0=csl,
                                                    in1=st[:],
                                                    op=mybir.AluOpType.max)
                        # row direction: accumulate max over jp into scratch
                        if jp == 0:
                            nc.vector.tensor_copy(out=rowscr[:], in_=st[:])
                        else:
                            nc.vector.tensor_tensor(out=rowscr[:],
                                                    in0=rowscr[:], in1=st[:],
                                                    op=mybir.AluOpType.max)
                    # per-row-block tail: fold once (fp16 2x TT) then reduce
                    h = PSUM_W // 2
                    nc.vector.tensor_tensor(out=rowhalf[:],
                                            in0=rowscr[:, 0:h],
                                            in1=rowscr[:, h:PSUM_W],
                                            op=mybir.AluOpType.max)
                    nc.vector.tensor_reduce(
                        out=rowcol[:, ib:ib + 1], in_=rowhalf[:],
                        axis=mybir.AxisListType.X, op=mybir.AluOpType.max)

            colred = sb.tile([MBLK, N], mybir.dt.float32)
            nc.gpsimd.partition_all_reduce(colred[:], colacc[:], channels=MBLK,
                                           reduce_op=bass_isa.ReduceOp.max)

            nc.sync.dma_start(row_d.ap()[:], rowcol[:])
            nc.sync.dma_start(col_d.ap()[:], colred[0:1, :])

    nc.compile()
    _NC_CACHE = nc
    return nc


def _split(v):
    """f32 -> (hi, lo) bf16 with v ~= hi + lo to ~18 mantissa bits."""
    hi = v.astype(BF16)
    lo = (v - hi.astype(np.float32)).astype(BF16)
    return hi, lo


def _prep_core(xc, yb):
    """Build the K=16 augmented bf16 operands for one core.

    xc: [HALF, 3] f32 x-chunk; yb: [N, 3] f32 full y for the batch.
    Row k of lhs/rhs multiply pairwise and accumulate so that
    psum[i, j] = -dist2(x_i, y_j) to ~1e-5 abs.
    """
    x2 = np.sum(xc * xc, axis=1)
    y2 = np.sum(yb * yb, axis=1)
    nx2h, nx2l = _split(-x2)
    ny2h, ny2l = _split(-y2)
    xh, xl = _split(xc)
    yh, yl = _split(yb)
    txh = (xh.astype(np.float32) * 2.0).astype(BF16)  # exact in bf16
    txl = (xl.astype(np.float32) * 2.0).astype(BF16)

    lhs = np.zeros((K, HALF), dtype=BF16)
    rhs = np.zeros((K, N), dtype=BF16)
    lhs[0] = nx2h
    lhs[1] = nx2l
    rhs[0] = np.ones(N, BF16)
    rhs[1] = np.ones(N, BF16)
    lhs[2] = np.ones(HALF, BF16)
    lhs[3] = np.ones(HALF, BF16)
    rhs[2] = ny2h
    rhs[3] = ny2l
    for d in range(D):
        lhs[4 + d] = txh[:, d]
        rhs[4 + d] = yh[:, d]
        lhs[7 + d] = txl[:, d]
        rhs[7 + d] = yh[:, d]
        lhs[10 + d] = txh[:, d]
        rhs[10 + d] = yl[:, d]
        lhs[13 + d] = txl[:, d]
        rhs[13 + d] = yl[:, d]
    return {"lhst": lhs, "rhs": rhs}


def make_in_maps(x, y):
    x = np.ascontiguousarray(np.asarray(x, dtype=np.float32))
    y = np.ascontiguousarray(np.asarray(y, dtype=np.float32))
    in_maps = []
    for c in range(NCORES):
        b, h = c // 2, c % 2
        in_maps.append(_prep_core(x[b, h * HALF:(h + 1) * HALF], y[b]))
    return in_maps


def combine(results):
    """results: list of 8 dicts with 'rowout' [128, 32] and 'colout' [1, N],
    both holding NEGATED maxes (i.e. -min distances)."""
    row_sum = 0.0
    col_sum = 0.0
    for b in range(B):
        r0 = results[2 * b]["rowout"].astype(np.float64)
        r1 = results[2 * b + 1]["rowout"].astype(np.float64)
        row_sum += -(r0.sum() + r1.sum())
        c0 = results[2 * b]["colout"][0]
        c1 = results[2 * b + 1]["colout"][0]
        col_sum += -np.maximum(c0, c1).astype(np.float64).sum()
    mean_x = row_sum / (B * N)
    mean_y = col_sum / (B * N)
    return np.asarray(mean_x + mean_y, dtype=np.float32)


def kernel(x, y):
    nc = _build_nc()
    in_maps = make_in_maps(x, y)
    res = run_bass_kernel_spmd(nc, in_maps, core_ids=list(range(NCORES)))
    return combine(res.results)
